# revision 41
# baseline (speedup 1.0000x reference)
"""Trainium2 Bass kernel for nn_MultiHeadAttention (SL=2048, BS=2, D=1024, H=16, DH=64).

Sharding: the [BS=2, H=16] grid of attention heads is split across 8 cores:
core c handles batch b = c//4 and heads 4*(c%4) .. 4*(c%4)+4.
Each core computes q/k/v projections for its own head slice, the 4 attention
maps, and a partial output (its heads' contribution through Wo). The host
sums the 4 partials per batch.

All matmuls run as float32r (fp32 storage, FP22 multiply) at full PE rate.
Scores are computed transposed (S^T[k, q]) so softmax-exp output feeds the
AV matmul directly; softmax denominators come from an ones-matmul
(column-sum over PSUM partitions), replicated across 64 partitions so the
normalization is a plain elementwise multiply.
"""

import os
import ml_dtypes
import numpy as np

SL, BS, D = 2048, 2, 1024
H, DH = 16, 64
NCORES = 8
HPC = 4            # heads per core
OD = HPC * DH      # 256 projected dims per core
DC = D // 128      # 8 contraction chunks
QC = SL // 512     # 4 query chunks of 512
KT = SL // 128     # 16 key tiles of 128

_NC = None
LAST_RESULT = None


def _build_nc():
    import concourse.mybir as mybir
    import concourse.tile as tile
    from concourse import bacc

    f32 = mybir.dt.float32
    f32r = mybir.dt.float32r
    bf16 = mybir.dt.bfloat16
    f16 = mybir.dt.float16
    EXP = mybir.ActivationFunctionType.Exp

    nc = bacc.Bacc(None, target_bir_lowering=False, debug=True)

    xqT = nc.dram_tensor("xqT", [D, SL], f16, kind="ExternalInput")
    xkT = nc.dram_tensor("xkT", [D, SL], f16, kind="ExternalInput")
    xvT = nc.dram_tensor("xvT", [D, SL], f16, kind="ExternalInput")
    wqT = nc.dram_tensor("wqT", [128, DC * OD], f16, kind="ExternalInput")
    wkT = nc.dram_tensor("wkT", [128, DC * OD], f16, kind="ExternalInput")
    wvT = nc.dram_tensor("wvT", [128, DC * OD], f16, kind="ExternalInput")
    woT = nc.dram_tensor("woT", [128, 2 * D], f16, kind="ExternalInput")
    onesvd = nc.dram_tensor("onesvd", [128, 260], bf16, kind="ExternalInput")
    yT = nc.dram_tensor("yT", [D, SL], f32, kind="ExternalOutput")
    debug = bool(int(os.environ.get("KERNEL_DEBUG", "0")))
    if debug:
        dbg_qT = [nc.dram_tensor(f"dbg_qT{i}", [128, SL], f16, kind="ExternalOutput") for i in range(2)]
        dbg_kT = [nc.dram_tensor(f"dbg_kT{i}", [128, SL], f16, kind="ExternalOutput") for i in range(2)]
        dbg_v0 = nc.dram_tensor("dbg_v0", [128, 260], bf16, kind="ExternalOutput")
        dbg_E = nc.dram_tensor("dbg_E", [128, 1024], bf16, kind="ExternalOutput")
        dbg_OT = nc.dram_tensor("dbg_OT", [128, 512], f32r, kind="ExternalOutput")
        dbg_AVO = [nc.dram_tensor(f"dbg_AVO{i}", [65, 512], f32, kind="ExternalOutput") for i in range(2)]
        dbg_BC = [nc.dram_tensor(f"dbg_BC{i}", [64, 512], f32, kind="ExternalOutput") for i in range(2)]
        dbg_rec = [nc.dram_tensor(f"dbg_rec{i}", [1, 512], f32, kind="ExternalOutput") for i in range(2)]
        dbg_recr = [nc.dram_tensor(f"dbg_recr{i}", [1, 512], f32r, kind="ExternalOutput") for i in range(2)]

    with tile.TileContext(nc) as tc:
        with (
            tc.tile_pool(name="wsb", bufs=1) as wsb,
            tc.tile_pool(name="qk", bufs=1) as qk,
            tc.tile_pool(name="vsb", bufs=1) as vsb,
            tc.tile_pool(name="xkp", bufs=16) as xkp,
            tc.tile_pool(name="xqp", bufs=16) as xqp,
            tc.tile_pool(name="xvp", bufs=16) as xvp,
            tc.tile_pool(name="esb", bufs=5) as esb,
            tc.tile_pool(name="rsb", bufs=2) as rsb,
            tc.tile_pool(name="otsb", bufs=4) as otsb,
            tc.tile_pool(name="ysb", bufs=4) as ysb,
            tc.tile_pool(name="otmp", bufs=2) as otmp,
            tc.tile_pool(name="avsb", bufs=2) as avsb,
            tc.tile_pool(name="pp", bufs=1, space="PSUM") as pp,
            tc.tile_pool(name="wp", bufs=2, space="PSUM") as wp,
            tc.tile_pool(name="avop", bufs=1, space="PSUM") as avop,
            tc.tile_pool(name="yp", bufs=1, space="PSUM") as yp,
        ):
            # --- persistent SBUF tensors ---
            wq_sb = wsb.tile([128, DC * OD], f16, tag="wq")  # [p, dc*256+od]
            wk_sb = wsb.tile([128, DC * OD], f16, tag="wk")
            wv_sb = wsb.tile([128, DC * OD], f16, tag="wv")
            wo_sb = wsb.tile([128, 2 * D], f16, tag="wo")    # [p, hp*1024+o]
            ones_sb = wsb.tile([128, 260], bf16, tag="ones")
            kT_sb = [qk.tile([128, SL], f16, tag=f"kT{ot}", name=f"kT{ot}") for ot in range(2)]
            qT_sb = [qk.tile([128, SL], f16, tag=f"qT{ot}", name=f"qT{ot}") for ot in range(2)]
            v_sb = [vsb.tile([128, 260], bf16, tag=f"v{t}", name=f"v{t}") for t in range(KT)]

            def load_w(dst, src):
                nc.sync.dma_start(out=dst[:], in_=src[:])

            def load_x(pool, tg, xdram, cc, dt_=f32r):
                tiles = []
                for d in range(DC):
                    t = pool.tile([128, 512], dt_, tag=tg, name="x")
                    nc.sync.dma_start(out=t[:],
                                      in_=xdram[d * 128:(d + 1) * 128,
                                                cc * 512:(cc + 1) * 512])
                    tiles.append(t[:])
                return tiles

            def load_x2(pool, tg, xdram, cc2, dt_=f32r, eng=None):
                # one [128, 1024] DMA per D-chunk: bigger descriptors, twice
                # the per-queue bandwidth; returns slice lists for both
                # 512-token halves
                ev, od = [], []
                trig = nc.scalar if eng == "scalar" else nc.sync
                for d in range(DC):
                    t = pool.tile([128, 1024], dt_, tag=tg, name="x")
                    trig.dma_start(out=t[:],
                                   in_=xdram[d * 128:(d + 1) * 128,
                                             cc2 * 1024:(cc2 + 1) * 1024])
                    ev.append(t[:, 0:512])
                    od.append(t[:, 512:1024])
                return ev, od

            qp_ps = {}

            def proj_qk_quarter(w_sb, dst, cc, xtiles, ot, half, alt=False):
                if half == 0:
                    # in the prologue (alt=True) alternate the PSUM bank so
                    # consecutive quarter-pairs don't serialize on the copy
                    pool, ptag = ((yp, "yp") if alt and (2 * cc + ot) % 2
                                  else (pp, "pp"))
                    qp_ps[(cc, ot)] = pool.tile([128, 512], f32, tag=ptag,
                                                name="ps")
                ps = qp_ps[(cc, ot)]
                for d in range(half * 4, half * 4 + 4):
                    nc.tensor.matmul(
                        ps[:],
                        (w_sb[:, d * OD + ot * 128: d * OD + (ot + 1) * 128]),
                        (xtiles[d][:]),
                        start=(d == 0), stop=(d == DC - 1))
                if half == 1:
                    nc.vector.tensor_copy(dst[ot][:, cc * 512:(cc + 1) * 512],
                                          ps[:])

            def proj_qk_half(w_sb, dst, cc, xtiles, ot, alt=False):
                proj_qk_quarter(w_sb, dst, cc, xtiles, ot, 0, alt)
                proj_qk_quarter(w_sb, dst, cc, xtiles, ot, 1, alt)

            def proj_qk(w_sb, dst, cc, xtiles, alt=False):
                for ot in range(2):
                    proj_qk_half(w_sb, dst, cc, xtiles, ot, alt)

            def emit_qproj(qc_):
                xt = load_x(xqT, qc_)
                proj_qk(wq_sb, qT_sb, qc_, xt)

            def emit_wo_piece(qc_, ot_tiles, pool, ptag, o8, copy_eng=None,
                              dma_eng="gpsimd"):
                Y = pool.tile([128, 512], f32, tag=ptag, name="Y")
                for hp in range(2):
                    nc.tensor.matmul(
                        Y[:],
                        (wo_sb[:, hp * D + o8 * 128: hp * D + (o8 + 1) * 128]),
                        (ot_tiles[hp][:]),
                        start=(hp == 0), stop=(hp == 1))
                ys = ysb.tile([128, 512], f32, tag="ys", name="ys")
                if copy_eng == "scalar":
                    nc.scalar.copy(ys[:], Y[:])
                else:
                    nc.vector.tensor_copy(ys[:], Y[:])
                trig = {"gpsimd": nc.gpsimd, "scalar": nc.scalar,
                        "sync": nc.sync}[dma_eng]
                trig.dma_start(
                    out=yT[o8 * 128:(o8 + 1) * 128, qc_ * 512:(qc_ + 1) * 512],
                    in_=ys[:])

            def emit_wo(qc_, ot_tiles, pools, alternate=False):
                for o8 in range(8):
                    pool, ptag = pools[o8 % len(pools)]
                    eng = "scalar" if (alternate and o8 % 2 == 1) else None
                    demg = ("scalar" if o8 % 2 == 1 else "sync") if alternate \
                        else "gpsimd"
                    emit_wo_piece(qc_, ot_tiles, pool, ptag, o8, eng, demg)

            # --- prologue: dual-queue DMA dispatch.  scalar queue carries
            # wk + xk (the critical k-proj inputs, at [128,1024] cc2
            # granularity so k-proj starts after the first 2.5MB); sync
            # carries everything else in consumption order.  gpsimd
            # pre-fills the ones columns of the v tiles. ---
            nc.scalar.dma_start(out=ones_sb[:], in_=onesvd[:])
            nc.scalar.dma_start(out=wk_sb[:], in_=wkT[:])
            xkA = load_x2(xkp, 'xk', xkT, 0, f16, eng="scalar")
            xkB = load_x2(xkp, 'xk', xkT, 1, f16, eng="scalar")
            nc.sync.dma_start(out=wq_sb[:], in_=wqT[:])
            xq_t = {0: load_x(xqp, 'xq', xqT, 0, f16)}
            nc.sync.dma_start(out=wv_sb[:], in_=wvT[:])
            _xv01 = load_x2(xvp, 'xv', xvT, 0, f16)
            xv_chunks = {0: _xv01[0], 1: _xv01[1]}
            for t_ in range(KT):
                nc.gpsimd.memset(v_sb[t_][:], 1.0)

            warm = yp.tile([128, 512], f32, tag="yp", name="warm")
            for i in range(24):
                nc.tensor.matmul(warm[0:64, 0:256], ones_sb[:, 0:64],
                                 ones_sb[:, 0:256], start=(i == 0),
                                 stop=(i == 23))
            warms = ysb.tile([64, 256], f32, tag="ys", name="warms")
            nc.vector.tensor_copy(warms[:], warm[0:64, 0:256])
            # k proj cc 0,1 from the first xk half, then q0 (its DMA lands
            # early on the sync queue) while the second xk half arrives
            proj_qk(wk_sb, kT_sb, 0, xkA[0], alt=True)
            proj_qk(wk_sb, kT_sb, 1, xkA[1], alt=True)
            proj_qk(wq_sb, qT_sb, 0, xq_t[0], alt=True)
            proj_qk(wk_sb, kT_sb, 2, xkB[0], alt=True)
            proj_qk(wk_sb, kT_sb, 3, xkB[1], alt=True)

            def emit_vtile(t_):
                cc_, tt = divmod(t_, 4)
                xtiles = xv_chunks[cc_]
                pool, ptag = (yp, "yp") if t_ % 2 else (pp, "pp")
                ps = pool.tile([128, OD], f32, tag=ptag, name="ps")
                for d in range(DC):
                    nc.tensor.matmul(
                        ps[:],
                        (xtiles[d][:, tt * 128:(tt + 1) * 128]),
                        (wv_sb[:, d * OD:(d + 1) * OD]),
                        start=(d == 0), stop=(d == DC - 1))
                for h in range(4):
                    nc.vector.tensor_copy(
                        v_sb[t_][:, h * 65:h * 65 + 64],
                        ps[:, h * 64:(h + 1) * 64])
            if debug:
                for i in range(2):
                    nc.sync.dma_start(out=dbg_kT[i][:], in_=kT_sb[i][:])
                nc.sync.dma_start(out=dbg_v0[:], in_=v_sb[0][:])

            # --- attention: 8 ladders (qc-major, head-pair minor), with
            # fine-grained insertions so ACT stays saturated ---
            inserts = {}

            def at(L_, kt_, fn):
                inserts.setdefault((L_, kt_), []).append(fn)

            OTs = {}

            # --- schedule: xq loads at ladder starts; q-proj quarters,
            # normalize chains and Wo pieces spread one-per-slot so every
            # slot's PE work stays just under the ACT exp floor ---
            # xv23 and wo loads deferred into L0 so the prologue's DMA
            # bandwidth goes entirely to the k/q-proj inputs
            def _load_xv23():
                ev, od = load_x2(xvp, 'xv', xvT, 1, f16)
                xv_chunks[2] = ev
                xv_chunks[3] = od
            at(0, 0, _load_xv23)
            at(0, 8, (lambda: nc.sync.dma_start(out=wo_sb[:], in_=woT[:])))

            for qc_ in range(1, QC):
                Lt = 2 * (qc_ - 1)
                at(Lt, 0 if qc_ > 1 else 4,
                   (lambda q=qc_: xq_t.__setitem__(q, load_x(xqp, 'xq', xqT, q, f16))))
                for j in range(4):
                    ot, hf = j // 2, j % 2
                    if qc_ == 1:
                        Ls, slot = 1, 8 + 2 * j
                    else:
                        Ls, slot = Lt + ot, 9 + 2 * hf
                    at(Ls, slot,
                       (lambda q=qc_, ot=ot, hf=hf:
                        proj_qk_quarter(wq_sb, qT_sb, q, xq_t[q], ot, hf)))

            # --- attention: one continuous 128-slot pipeline.  Slot s does
            # scores+exp for (L,kt)=divmod(s,16) and the AV for slot s-4, so
            # each ladder's first scores overlap the previous ladder's
            # trailing AVs and the ACT engine never drains at boundaries.
            # AVO accumulators are allocated lazily (at each ladder's first
            # AV) so the pool WAR lands after the previous evacuation. ---
            LAG = 4
            pend = []
            AVOs = {}

            def finish_ladder(Lp):
                qcp, hpp = divmod(Lp, 2)
                last = (Lp == 2 * QC - 1)
                avs_pair = []
                for hip in range(2):
                    avs = avsb.tile([65, 512], f32, tag="avs", name="avs")
                    if last and hip == 1:
                        nc.scalar.copy(avs[:], AVOs[Lp][hip][:])
                    else:
                        nc.vector.tensor_copy(avs[:], AVOs[Lp][hip][:])
                    avs_pair.append(avs)
                OT = otsb.tile([128, 512], f16, tag="ot", name="OT")
                OTs[(qcp, hpp)] = OT

                def chain(hip, avs_pair=avs_pair, OT=OT):
                    avs = avs_pair[hip]
                    sums_r = rsb.tile([65, 512], bf16, tag="recip", name="sums_r")
                    nc.vector.tensor_copy(sums_r[64:65, :], avs[64:65, :])
                    BCp = pp.tile([64, 512], f32, tag="pp", name="BCp")
                    nc.tensor.matmul(BCp[:], ones_sb[64:65, 0:64],
                                     sums_r[64:65, :], start=True, stop=True)
                    sumsb = ysb.tile([64, 512], f32, tag="ys", name="sumsb")
                    nc.vector.tensor_copy(sumsb[:], BCp[:])
                    BCs = ysb.tile([64, 512], f32, tag="ys", name="BCs")
                    nc.vector.reciprocal_approx_fast(BCs[:], sumsb[:])
                    if hip == 0:
                        nc.vector.tensor_mul(OT[0:64, :], avs[0:64, :], BCs[:])
                    else:
                        OTt = otmp.tile([64, 512], f16, tag="otmp", name="OTt")
                        nc.vector.tensor_mul(OTt[:], avs[0:64, :], BCs[:])
                        nc.sync.dma_start(out=OT[64:128, :], in_=OTt[:])

                def tail_chains(avs_pair=avs_pair, OT=OT):
                    # normalize both hips with the hip1 sum path on the idle
                    # ACT engine and everything else interleaved on DVE so
                    # the two chains overlap
                    sums1 = rsb.tile([65, 512], bf16, tag="recip", name="s1")
                    nc.scalar.copy(sums1[64:65, :], avs_pair[1][64:65, :])
                    sums0 = rsb.tile([65, 512], bf16, tag="recip", name="s0")
                    nc.vector.tensor_copy(sums0[64:65, :],
                                          avs_pair[0][64:65, :])
                    BCp1 = yp.tile([64, 512], f32, tag="yp", name="BCp1")
                    nc.tensor.matmul(BCp1[:], ones_sb[64:65, 0:64],
                                     sums1[64:65, :], start=True, stop=True)
                    BCp0 = pp.tile([64, 512], f32, tag="pp", name="BCp0")
                    nc.tensor.matmul(BCp0[:], ones_sb[64:65, 0:64],
                                     sums0[64:65, :], start=True, stop=True)
                    # keep the PE's p-state up through the serial normalize
                    # chain so the Wo matmuls below run at full clock
                    keep = wp.tile([128, 1024], f32, tag="wp", name="keep")
                    for i in range(28):
                        nc.tensor.matmul(keep[0:64, 0:256],
                                         ones_sb[:, 0:64], ones_sb[:, 0:256],
                                         start=(i == 0), stop=(i == 27))
                    sumsb1 = ysb.tile([64, 512], f32, tag="ys", name="sb1")
                    nc.scalar.copy(sumsb1[:], BCp1[:])
                    sumsb0 = ysb.tile([64, 512], f32, tag="ys", name="sb0")
                    nc.vector.tensor_copy(sumsb0[:], BCp0[:])
                    # hip1 first: its path ends in the sbuf-to-sbuf DMA hop,
                    # so get that in flight before normalizing hip0
                    BCs1 = ysb.tile([64, 512], f32, tag="ys", name="BCs1")
                    nc.vector.reciprocal_approx_fast(BCs1[:], sumsb1[:])
                    OTt = otmp.tile([64, 512], f16, tag="otmp", name="OTt")
                    nc.vector.tensor_mul(OTt[:], avs_pair[1][0:64, :], BCs1[:])
                    nc.scalar.dma_start(out=OT[64:128, :], in_=OTt[:])
                    BCs0 = ysb.tile([64, 512], f32, tag="ys", name="BCs0")
                    nc.vector.reciprocal_approx_fast(BCs0[:], sumsb0[:])
                    nc.vector.tensor_mul(OT[0:64, :], avs_pair[0][0:64, :],
                                         BCs0[:])
                    keeps = ysb.tile([64, 256], f32, tag="ys", name="keeps")
                    nc.vector.tensor_copy(keeps[:], keep[0:64, 0:256])

                if not last:
                    at(Lp + 1, 4, (lambda c=chain: c(0)))
                    at(Lp + 1, 6, (lambda c=chain: c(1)))
                    # Wo pieces for qcp: 0-3 on ladder Lp+1, 4-7 on Lp+2
                    if hpp == 1 and qcp < QC - 1:
                        for j, o8 in enumerate(range(4)):
                            at(Lp + 1, (8, 10, 13, 15)[j],
                               (lambda q=qcp, o=o8:
                                emit_wo_piece(q, [OTs[(q, 0)], OTs[(q, 1)]],
                                              yp, 'yp', o)))
                        for j, o8 in enumerate(range(4, 8)):
                            at(Lp + 2, (8, 10, 13, 15)[j],
                               (lambda q=qcp, o=o8:
                                emit_wo_piece(q, [OTs[(q, 0)], OTs[(q, 1)]],
                                              yp, 'yp', o)))
                else:
                    tail_chains()
                    emit_wo(qcp, [OTs[(qcp, 0)], OTs[(qcp, 1)]],
                            [(yp, 'yp'), (pp, 'pp'),
                             (avop, 'av0'), (avop, 'av1')],
                            alternate=True)

            def emit_av2(E_, Lp, ktp):
                qcp, hpp = divmod(Lp, 2)
                if ktp == 0:
                    AVOs[Lp] = [avop.tile([65, 512], f32, tag=f"av{hip}",
                                          name="AVO") for hip in range(2)]
                for hip in range(2):
                    nc.tensor.matmul(
                        AVOs[Lp][hip][:],
                        (v_sb[ktp][:, (hpp * 2 + hip) * 65:
                                     (hpp * 2 + hip) * 65 + 65]),
                        (E_[:, hip * 512:(hip + 1) * 512]),
                        start=(ktp == 0), stop=(ktp == KT - 1))
                if ktp == KT - 1:
                    finish_ladder(Lp)
                    AVOs.pop(Lp)

            for s in range(2 * QC * KT + LAG):
                if s < 2 * QC * KT:
                    L, kt = divmod(s, KT)
                    qc, hp = divmod(L, 2)
                    W = wp.tile([128, 1024], f32, tag="wp", name="W")
                    for hip in range(2):
                        nc.tensor.matmul(
                            W[:, hip * 512:(hip + 1) * 512],
                            (kT_sb[hp][hip * 64:(hip + 1) * 64,
                                         kt * 128:(kt + 1) * 128]),
                            (qT_sb[hp][hip * 64:(hip + 1) * 64,
                                         qc * 512:(qc + 1) * 512]),
                            start=True, stop=True)
                    E = esb.tile([128, 1024], bf16, tag="E", name="E")
                    nc.scalar.activation(E[:], W[:], EXP)
                    if debug and L == 0 and kt == 0:
                        nc.sync.dma_start(out=dbg_E[:], in_=E[:])
                        for i in range(2):
                            nc.sync.dma_start(out=dbg_qT[i][:, 0:512],
                                              in_=qT_sb[i][:, 0:512])
                    pend.append((E, L, kt))
                if len(pend) > LAG or s >= 2 * QC * KT:
                    pE, pL, pkt = pend.pop(0)
                    emit_av2(pE, pL, pkt)
                if s < 2 * QC * KT:
                    # v tiles ride 3 slots behind their L0 slot (the AV lag
                    # covers 4) so the xv DMAs can be deferred
                    if L == 0 and kt >= 3:
                        emit_vtile(kt - 3)
                    elif L == 1 and kt <= 2:
                        emit_vtile(13 + kt)
                    for fn in inserts.pop((L, kt), []):
                        fn()

    nc.compile()
    return nc


def _get_nc():
    global _NC
    if _NC is None:
        _NC = _build_nc()
    return _NC


def _host_fallback(query, keys, values, mask, Wq, Wk, Wv, Wo):
    # Exact reference math in numpy; only used if mask has zeros (off-spec).
    q = (query @ Wq.T).reshape(SL, BS, H, DH)
    k = (keys @ Wk.T).reshape(SL, BS, H, DH)
    v = (values @ Wv.T).reshape(SL, BS, H, DH)
    out = np.zeros((SL, BS, H * DH), np.float32)
    for b in range(BS):
        for h in range(H):
            s = q[:, b, h, :] @ k[:, b, h, :].T
            s = np.where(mask[0, 0] == 0, np.float32(-1e20), s)
            s = s - s.max(axis=-1, keepdims=True)
            p = np.exp(s)
            p /= p.sum(axis=-1, keepdims=True)
            out[:, b, h * DH:(h + 1) * DH] = p @ v[:, b, h, :]
    return out @ Wo.T


def _enable_trace_support():
    """Install the antenv.axon_hooks shim so trace=True works under axon."""
    import sys
    import types
    import antenv
    if "antenv.axon_hooks" in sys.modules:
        return
    hookmod = types.ModuleType("antenv.axon_hooks")
    _hook = [None]
    hookmod.set_axon_ntff_profile_hook = lambda h: _hook.__setitem__(0, h)
    hookmod.get_axon_ntff_profile_hook = lambda: _hook[0]
    antenv.axon_hooks = hookmod
    sys.modules["antenv.axon_hooks"] = hookmod
    try:
        from trn_agent_boot.trn_boot import _ntff_profile_via_ctypes
        hookmod.set_axon_ntff_profile_hook(
            _ntff_profile_via_ctypes("/opt/axon/libaxon_pjrt.so"))
    except Exception:
        pass
    import concourse.bass_utils as bu
    bu.upload_artifacts = lambda tmpdir: tmpdir


def _w_sb_layout(Wslice):
    # [256 od, 1024 D] -> [128 p, dc*256+od]
    return np.ascontiguousarray(
        Wslice.reshape(OD, DC, 128).transpose(2, 1, 0).reshape(128, DC * OD))


def _wo_sb_layout(WoSlice):
    # [1024 o, 256 hd] -> [128 p, hp*1024+o]
    return np.ascontiguousarray(
        WoSlice.reshape(D, 2, 128).transpose(2, 1, 0).reshape(128, 2 * D))


def kernel(query, keys, values, mask, Wq, Wk, Wv, Wo):
    query = np.asarray(query, np.float32)
    keys = np.asarray(keys, np.float32)
    values = np.asarray(values, np.float32)
    mask = np.asarray(mask)
    Wq = np.asarray(Wq, np.float32)
    Wk = np.asarray(Wk, np.float32)
    Wv = np.asarray(Wv, np.float32)
    Wo = np.asarray(Wo, np.float32)

    if (mask == 0).any():
        return _host_fallback(query, keys, values, mask, Wq, Wk, Wv, Wo)

    trace = bool(int(os.environ.get("KERNEL_TRACE", "0")))
    if trace:
        _enable_trace_support()

    from concourse.bass_utils import run_bass_kernel_spmd

    nc = _get_nc()
    in_maps = []
    for c in range(NCORES):
        b, hg = divmod(c, 4)
        hs = hg * OD
        in_maps.append({
            "xqT": np.ascontiguousarray(query[:, b, :].T).astype(np.float16),
            "xkT": np.ascontiguousarray(keys[:, b, :].T).astype(np.float16),
            "xvT": np.ascontiguousarray(values[:, b, :].T).astype(np.float16),
            "wqT": _w_sb_layout(Wq[hs:hs + OD, :]).astype(np.float16),
            "wkT": _w_sb_layout(Wk[hs:hs + OD, :]).astype(np.float16),
            "wvT": _w_sb_layout(Wv[hs:hs + OD, :]).astype(np.float16),
            "woT": _wo_sb_layout(Wo[:, hs:hs + OD]).astype(np.float16),
            "onesvd": np.ones((128, 260), ml_dtypes.bfloat16),
        })

    res = run_bass_kernel_spmd(nc, in_maps, core_ids=list(range(NCORES)),
                               trace=trace)
    global LAST_RESULT
    LAST_RESULT = res

    out = np.zeros((SL, BS, D), np.float32)
    for c in range(NCORES):
        b = c // 4
        out[:, b, :] += res.results[c]["yT"].T
    return out

